# revision 15
# baseline (speedup 1.0000x reference)
"""Trainium2 Bass kernel for AxialMHA (B=2, N=2048, D=1024, H=16, dh=64).

Sharding: tensor-parallel over heads - 16 heads / 8 cores = 2 heads per core.
Each core computes q/k/v projections for its 2 heads (full batch), runs
attention, and produces a partial output projection (contraction over its
128 feature dims). Host sums the 8 bf16 partials and adds the effective bias
(bv @ Wproj + bproj; the k-bias is dropped entirely - q.bk is constant per
query so it cancels in softmax).

PE plan: QKV projections in bf16 (full 128-contraction, 1 cyc/row); scores
and AV in fp8e4 DoubleRow (0.5 cyc/row, two contraction slots per
instruction).  Scores split the dh=64 contraction as 2x32 DoubleRow slots;
Q/K are written fp8 to [128, tok] staging tiles then DMA-restructured
(SBUF->SBUF) into [32 part, dh-half, head, tok].  AV pairs adjacent k-token
tiles in the two slots; lhsT = [ones(64)|V_h] so softmax denominators ride
along on psum rows 0:64.

exp is split across ACT (exact Exp, fp8 out) and DVE (one tensor_scalar
building the e4m3 bit pattern directly: u8 = trunc(s*log2e + 56.5-8c),
bitcast to fp8; the log-domain sawtooth error mostly cancels through the
softmax normalization).  GpSimd cannot touch PSUM on TRN2 so all
PSUM-reading work lives on ACT/DVE, balanced via the *PAT tables.
"""

import os
import sys

import numpy as np
import ml_dtypes

for _p in ("/opt/trn_rl_repo",):
    if _p not in sys.path and os.path.isdir(_p):
        sys.path.insert(0, _p)

import concourse.bass as bass
import concourse.tile as tile
from concourse import bacc, mybir
from concourse.bass_utils import run_bass_kernel_spmd

BF16 = mybir.dt.bfloat16
F32 = mybir.dt.float32
FP8 = mybir.dt.float8e4
U8 = mybir.dt.uint8
AF = mybir.ActivationFunctionType
ALU = mybir.AluOpType
DR = mybir.MatmulPerfMode.DoubleRow

B, N, D, H, DH = 2, 2048, 1024, 16, 64
NC = 8            # cores
HC = H // NC      # heads per core = 2
TOK = B * N       # 4096
CH = 8            # token chunks of 512 for projections
CW = TOK // CH    # 512
KTD = D // 128    # 8 contraction tiles for projections
NKT = N // 128    # 16 ktok tiles per batch
QC = N // 512     # 4 qchunks per batch

# int8-exp constants: u8 = trunc(s*LOG2E + MAGIC) built directly as e4m3
# bits; logits s/8 stay within +-2.2 so u8 stays in [32, 80].
EXP_MUL = float(np.log2(np.e))            # folds the 1/8 logit scale
EXP_ADD = 56.03676                        # calibrated so the int8-exp is
                                          # mean-unbiased vs the ACT path

# engine assignment patterns (A=ACT, D=DVE), tuned for makespan balance
EXPPAT = "ADADADADADADADAA"               # phase 2: A9 D7, no runs
EXPPAT3 = "ADADADADADADADAA"              # phase 3: A9 D7, no runs               # per (b,qc): 16 (ktg,h) slots
COPYPAT = "ADAADAAD"                      # A5 D3 per chunk                      # proj psum->sbuf copies per chunk


def build_nc():
    nc = bacc.Bacc(
        "TRN2",
        target_bir_lowering=False,
        debug=False,
        enable_asserts=False,
        num_devices=NC,
    )
    xT = nc.dram_tensor("xT", [D, TOK], BF16, kind="ExternalInput").ap()
    wq = nc.dram_tensor("wq", [D, 128], BF16, kind="ExternalInput").ap()
    wk = nc.dram_tensor("wk", [D, 128], BF16, kind="ExternalInput").ap()
    wv = nc.dram_tensor("wv", [D, 128], BF16, kind="ExternalInput").ap()
    wo = nc.dram_tensor("wo", [128, D], BF16, kind="ExternalInput").ap()
    bq = nc.dram_tensor("bq", [128, 1], F32, kind="ExternalInput").ap()
    out_p = nc.dram_tensor("out_p", [D, TOK], BF16, kind="ExternalOutput").ap()

    from contextlib import ExitStack

    with tile.TileContext(nc) as tc, ExitStack() as ctx:
        singles = ctx.enter_context(tc.tile_pool(name="singles", bufs=1))

        wq_sb = singles.tile([128, KTD, 128], BF16)
        nc.sync.dma_start(wq_sb, wq.rearrange("(ko p) m -> p ko m", p=128))
        wk_sb = singles.tile([128, KTD, 128], BF16)
        wv_sb = singles.tile([128, KTD, 128], BF16)
        bq_sb = singles.tile([128, 1], F32)
        wo_sb = singles.tile([128, D], BF16)

        # fp8 staging for Q/K (d-on-partitions), then DMA-restructured into
        # [32 part, dh-half, head, tok] for DoubleRow scores
        QT = [singles.tile([128, N], FP8, name=f"QT{b}") for b in range(B)]
        KT = [singles.tile([128, N], FP8, name=f"KT{b}") for b in range(B)]
        Q2 = [singles.tile([32, 2, HC, N], FP8, name=f"Q2{b}") for b in range(B)]
        K2 = [singles.tile([32, 2, HC, N], FP8, name=f"K2{b}") for b in range(B)]
        # token-major V per (b, kchunk): cols 0:64 ones (denominator rows),
        # cols 64:128 = V_h
        V1 = [[singles.tile([128, 4, HC, 2 * DH], FP8, name=f"V1{b}_{q}")
               for q in range(QC)] for b in range(B)]
        for b in range(B):
            for q in range(QC):
                nc.gpsimd.memset(V1[b][q][:, :, :, 0:DH], 1.0)
        yT = [singles.tile([128, N], BF16, name=f"yT{b}") for b in range(B)]

        # ---- Stage A: bf16 QKV projections for one 512-token chunk.
        # Emitted as a generator with 8 interleave slots so attention ktg
        # blocks can be zipped between the PE matmul groups (in-order engine
        # queues: emission order = execution order). pq occupies slots 0-3,
        # pk slots 4-7 (they share one psum bank), pv spreads k over all 8.
        def stage_a_steps(b, cc, xpool, psA, psV):
            c = b * (CH // B) + cc
            xt = xpool.tile([128, KTD, CW], BF16, tag="xt", name="xt")
            xsrc = xT[:, c * CW:(c + 1) * CW].rearrange("(ko p) n -> p ko n", p=128)
            nc.sync.dma_start(xt[:, 0:KTD // 2, :], xsrc[:, 0:KTD // 2, :])
            nc.sync.dma_start(xt[:, KTD // 2:, :], xsrc[:, KTD // 2:, :])
            if b == 0 and cc == 0:
                nc.sync.dma_start(wk_sb, wk.rearrange("(ko p) m -> p ko m", p=128))
                nc.sync.dma_start(wv_sb, wv.rearrange("(ko p) m -> p ko m", p=128))
                nc.sync.dma_start(bq_sb, bq)
                nc.sync.dma_start(wo_sb, wo)
            bs = slice(c * CW - b * N, (c + 1) * CW - b * N)
            pq = psA.tile([128, CW], F32, tag="pqk", name="pq")
            pv = psV.tile([128, 4, 128], F32, tag="pv", name="pv")
            pk = None
            for slot in range(8):
                if slot < 4:
                    for kk in (2 * slot, 2 * slot + 1):
                        nc.tensor.matmul(pq, lhsT=wq_sb[:, kk, :], rhs=xt[:, kk, :],
                                         start=(kk == 0), stop=(kk == KTD - 1))
                else:
                    if pk is None:
                        pk = psA.tile([128, CW], F32, tag="pqk", name="pk")
                    for kk in (2 * (slot - 4), 2 * (slot - 4) + 1):
                        nc.tensor.matmul(pk, lhsT=wk_sb[:, kk, :], rhs=xt[:, kk, :],
                                         start=(kk == 0), stop=(kk == KTD - 1))
                # pv s-subtiles share one psum bank; start=True flags the
                # whole 2KB region pending-zero, so each s must fully finish
                # (k 0..7) before the next s starts: s-major emission
                s = slot // 2
                for kk in range(4 * (slot % 2), 4 * (slot % 2) + 4):
                    nc.tensor.matmul(pv[:, s, :],
                                     lhsT=xt[:, kk, s * 128:(s + 1) * 128],
                                     rhs=wv_sb[:, kk, :],
                                     start=(kk == 0), stop=(kk == KTD - 1))
                if slot == 3:
                    # q bias-add + fp8 quantize on ACT (frees the pqk bank)
                    nc.scalar.activation(QT[b][:, bs], pq, AF.Identity,
                                         bias=bq_sb[:, 0:1])
                yield
            nc.vector.tensor_copy(KT[b][:, bs], pk)
            nc.scalar.copy(V1[b][cc][:, :, 0, DH:2 * DH], pv[:, :, 0:DH])
            nc.vector.tensor_copy(V1[b][cc][:, :, 1, DH:2 * DH],
                                  pv[:, :, DH:2 * DH])
            restructure_chunk(b, cc)

        def stage_a(b, cc, xpool, psA, psV):
            for _ in stage_a_steps(b, cc, xpool, psA, psV):
                pass

        # ---- per-chunk Q/K restructure to [32, half, head, tok] ----
        def restructure_chunk(b, cc):
            bs = slice(cc * CW, (cc + 1) * CW)
            for h in range(HC):
                for s in range(2):
                    ps = slice(h * 64 + 32 * s, h * 64 + 32 * (s + 1))
                    nc.sync.dma_start(Q2[b][:, s, h, bs], QT[b][ps, bs])
                    nc.sync.dma_start(K2[b][:, s, h, bs], KT[b][ps, bs])

        # ---- Output projection partial for one 512-token chunk ----
        def proj_open(b, cc, ppool):
            return ppool.tile([128, D // 128, CW], BF16, tag="ps", name="ps")

        def proj_ot(b, cc, ot, ps, psV, psA):
            cs = slice(cc * CW, (cc + 1) * CW)
            pool_, tag_ = (psV, "pv") if ot % 2 == 0 else (psA, "pqk")
            pp = pool_.tile([128, CW], F32, tag=tag_, name="pp")
            nc.tensor.matmul(pp, lhsT=wo_sb[:, ot * 128:(ot + 1) * 128],
                             rhs=yT[b][:, cs], start=True, stop=True)
            if COPYPAT[ot] == "A":
                nc.scalar.copy(ps[:, ot, :], pp)
            else:
                nc.vector.tensor_copy(ps[:, ot, :], pp)

        def proj_close(b, cc, ps):
            nc.sync.dma_start(
                out_p[:, b * N + cc * CW:b * N + (cc + 1) * CW]
                .rearrange("(o p) n -> p o n", p=128), ps)

        # ---- Attention pieces for one (batch, qchunk) ----
        def attn_open(yps):
            return [yps.tile([128, 512], F32, tag=f"y{h}", name=f"py{h}")
                    for h in range(HC)]

        def attn_se_h(b, qc, ktg, h, stp, epool, pat):
            # scores + exp for one (ktg, head); separate emission per head
            # staggers the two stt psum buffers against each other
            qs = slice(qc * 512, qc * 512 + 512)
            stt = stp.tile([128, 2, 512], F32, tag="st", name="stt")
            for j in range(2):
                kt = ktg * 2 + j
                nc.tensor.matmul(
                    stt[:, j, :],
                    lhsT=K2[b][:, :, h, kt * 128:(kt + 1) * 128],
                    rhs=Q2[b][:, :, h, qs],
                    start=True, stop=True, perf_mode=DR)
            et = epool.tile([128, 2, 512], FP8, tag="et", name="et")
            if pat[ktg * 2 + h] == "A":
                nc.scalar.activation(et, stt, AF.Exp, scale=0.125)
            else:
                nc.vector.tensor_scalar(
                    et.bitcast(U8), stt, EXP_MUL, EXP_ADD,
                    ALU.mult, ALU.add)
            return et

        def attn_av_h(b, ktg, h, py, et):
            kc, ks = divmod(ktg * 2, 4)
            nc.tensor.matmul(
                py[h],
                lhsT=V1[b][kc][:, ks:ks + 2, h, :],
                rhs=et,
                start=(ktg == 0), stop=(ktg == NKT // 2 - 1),
                perf_mode=DR)

        def attn_close(b, qc, py, epool):
            qs = slice(qc * 512, qc * 512 + 512)
            for h in range(HC):
                rsb = epool.tile([64, 512], F32, tag="den", name="rsb")
                nc.vector.reciprocal(rsb, py[h][0:DH, :])
                nc.vector.tensor_tensor(yT[b][h * DH:(h + 1) * DH, qs],
                                        py[h][DH:2 * DH, :],
                                        rsb, ALU.mult)

        with tc.tile_pool(name="xp", bufs=4) as xpool, \
             tc.tile_pool(name="psA", bufs=1, space="PSUM") as psA, \
             tc.tile_pool(name="psV", bufs=1, space="PSUM") as psV, \
             tc.tile_pool(name="stp", bufs=2, space="PSUM") as stp, \
             tc.tile_pool(name="yps", bufs=1, space="PSUM") as yps, \
             tc.tile_pool(name="ep", bufs=8) as epool, \
             tc.tile_pool(name="pp", bufs=2) as ppool:

            class AttIter:
                """One attention (batch, qchunk) iteration; emits scores/exp
                blocks on demand with the AVs auto-trailing by LAG blocks, so
                a new iteration's exp work can pre-run while the previous
                iteration's AVs and normalize drain."""

                LAG = 4   # in (ktg, h) half-blocks

                def __init__(self, b, qc, pat):
                    self.b, self.qc, self.pat = b, qc, pat
                    self.py = None
                    self.pend = []
                    self.i = 0

                def half(self):
                    if self.py is None:
                        self.py = attn_open(yps)
                    ktg, h = divmod(self.i, HC)
                    et = attn_se_h(self.b, self.qc, ktg, h, stp, epool, self.pat)
                    self.pend.append((ktg, h, et))
                    self.i += 1
                    while len(self.pend) > self.LAG:
                        k, hh, e = self.pend.pop(0)
                        attn_av_h(self.b, k, hh, self.py, e)

                def se(self, n):
                    # n counts full ktg blocks = 2 half-blocks
                    return [self.half] * (2 * n)

                blocks = se

                def close(self):
                    for k, hh, e in self.pend:
                        attn_av_h(self.b, k, hh, self.py, e)
                    self.pend = []
                    attn_close(self.b, self.qc, self.py, epool)

            class ProjIter:
                def __init__(self, b, cc):
                    self.b, self.cc = b, cc
                    self.ps = None
                    self.ot = 0

                def one(self):
                    if self.ps is None:
                        self.ps = proj_open(self.b, self.cc, ppool)
                    proj_ot(self.b, self.cc, self.ot, self.ps, psV, psA)
                    self.ot += 1

                def ots(self, n):
                    return [self.one] * n

                def close(self):
                    proj_close(self.b, self.cc, self.ps)

            def zipg(stage, work):
                """Interleave one stage chunk's 8 slots with work closures."""
                n = len(work)
                i = 0
                for s in range(8):
                    if stage is not None:
                        next(stage)
                    take = (n * (s + 1)) // 8 - (n * s) // 8
                    for _ in range(take):
                        work[i]()
                        i += 1
                if stage is not None:
                    for _ in stage:
                        pass

            att = {(b, qc): AttIter(b, qc, EXPPAT if b == 0 else EXPPAT3)
                   for b in range(B) for qc in range(QC)}
            prj = {(b, cc): ProjIter(b, cc) for b in range(B) for cc in range(QC)}
            S = lambda b, cc: stage_a_steps(b, cc, xpool, psA, psV)

            a00, a01, a02, a03 = (att[(0, q)] for q in range(QC))
            a10, a11, a12, a13 = (att[(1, q)] for q in range(QC))
            p00, p01, p02, p03 = (prj[(0, c)] for c in range(QC))
            p10, p11, p12, p13 = (prj[(1, c)] for c in range(QC))

            zipg(S(0, 0), [])
            zipg(S(0, 1), a00.se(2))
            zipg(S(0, 2), a00.se(2))
            zipg(S(0, 3), a00.se(2))
            zipg(S(1, 0), a00.se(2) + [a00.close] + a01.se(4))
            zipg(S(1, 1), a01.se(4) + [a01.close] + p00.ots(4) + a02.se(2))
            zipg(S(1, 2), a02.se(4) + p00.ots(4) + [p00.close]
                 + a02.se(2) + [a02.close])
            zipg(S(1, 3), a03.se(6) + p01.ots(4))
            zipg(None, a03.se(2) + [a03.close] + p01.ots(4) + [p01.close]
                 + a10.se(6) + p02.ots(4))
            zipg(None, a10.se(2) + [a10.close] + p02.ots(4) + [p02.close]
                 + a11.se(6) + p03.ots(4))
            zipg(None, a11.se(2) + [a11.close] + p03.ots(4) + [p03.close]
                 + a12.se(6) + p10.ots(4))
            zipg(None, a12.se(2) + [a12.close] + p10.ots(4) + [p10.close]
                 + a13.se(6) + p11.ots(4))
            zipg(None, a13.se(2) + [a13.close] + p11.ots(4) + [p11.close]
                 + p12.ots(8) + [p12.close])
            zipg(None, p13.ots(8) + [p13.close])

    nc.compile()
    return nc


_CACHE = {}


def _get_nc():
    if "nc" not in _CACHE:
        _CACHE["nc"] = build_nc()
    return _CACHE["nc"]


def _prep_inputs(x, Wqkv, bqkv, Wproj):
    bf = ml_dtypes.bfloat16
    x = np.asarray(x, np.float32)
    Wqkv = np.asarray(Wqkv, np.float32)
    bqkv = np.asarray(bqkv, np.float32)
    Wproj = np.asarray(Wproj, np.float32)
    xT = np.ascontiguousarray(x.reshape(TOK, D).T).astype(bf)
    in_maps = []
    for c in range(NC):
        m = {
            "xT": xT,
            "wq": np.ascontiguousarray(
                Wqkv[:, 0 * D + c * 128:0 * D + (c + 1) * 128]).astype(bf),
            "wk": np.ascontiguousarray(
                Wqkv[:, 1 * D + c * 128:1 * D + (c + 1) * 128]).astype(bf),
            "wv": np.ascontiguousarray(
                Wqkv[:, 2 * D + c * 128:2 * D + (c + 1) * 128]).astype(bf),
            "bq": np.ascontiguousarray(
                bqkv[c * 128:(c + 1) * 128]).reshape(128, 1).astype(np.float32),
            "wo": np.ascontiguousarray(Wproj[c * 128:(c + 1) * 128, :]).astype(bf),
        }
        in_maps.append(m)
    return in_maps


def _run(x, Wqkv, bqkv, Wproj, bproj, trace=False):
    Wproj = np.asarray(Wproj, np.float32)
    bproj = np.asarray(bproj, np.float32)
    bqkv_np = np.asarray(bqkv, np.float32)
    in_maps = _prep_inputs(x, Wqkv, bqkv_np, Wproj)
    nc = _get_nc()
    res = run_bass_kernel_spmd(nc, in_maps, core_ids=list(range(NC)), trace=trace)
    acc = res.results[0]["out_p"].astype(np.float32).copy()
    for c in range(1, NC):
        acc += res.results[c]["out_p"].astype(np.float32)
    bv = bqkv_np[2 * D:]
    bias_eff = (bv @ Wproj + bproj).astype(np.float32)
    out = np.ascontiguousarray(acc.T).reshape(B, N, D) + bias_eff
    return out.astype(np.float32), res


def kernel(x, Wqkv, bqkv, Wproj, bproj):
    out, _ = _run(x, Wqkv, bqkv, Wproj, bproj, trace=False)
    return out


# revision 16
# speedup vs baseline: 1.0286x; 1.0286x over previous
"""Trainium2 Bass kernel for AxialMHA (B=2, N=2048, D=1024, H=16, dh=64).

Sharding: tensor-parallel over heads - 16 heads / 8 cores = 2 heads per core.
Each core computes q/k/v projections for its 2 heads (full batch), runs
attention, and produces a partial output projection (contraction over its
128 feature dims). Host sums the 8 bf16 partials and adds the effective bias
(bv @ Wproj + bproj; the k-bias is dropped entirely - q.bk is constant per
query so it cancels in softmax).

PE plan: QKV projections in bf16 (full 128-contraction, 1 cyc/row); scores
and AV in fp8e4 DoubleRow (0.5 cyc/row, two contraction slots per
instruction).  Scores split the dh=64 contraction as 2x32 DoubleRow slots;
Q/K are written fp8 to [128, tok] staging tiles then DMA-restructured
(SBUF->SBUF) into [32 part, dh-half, head, tok].  AV pairs adjacent k-token
tiles in the two slots; lhsT = [ones(64)|V_h] so softmax denominators ride
along on psum rows 0:64.

exp is split across ACT (exact Exp, fp8 out) and DVE (one tensor_scalar
building the e4m3 bit pattern directly: u8 = trunc(s*log2e + 56.5-8c),
bitcast to fp8; the log-domain sawtooth error mostly cancels through the
softmax normalization).  GpSimd cannot touch PSUM on TRN2 so all
PSUM-reading work lives on ACT/DVE, balanced via the *PAT tables.
"""

import os
import sys

import numpy as np
import ml_dtypes

for _p in ("/opt/trn_rl_repo",):
    if _p not in sys.path and os.path.isdir(_p):
        sys.path.insert(0, _p)

import concourse.bass as bass
import concourse.tile as tile
from concourse import bacc, mybir
from concourse.bass_utils import run_bass_kernel_spmd

BF16 = mybir.dt.bfloat16
F32 = mybir.dt.float32
FP8 = mybir.dt.float8e4
U8 = mybir.dt.uint8
AF = mybir.ActivationFunctionType
ALU = mybir.AluOpType
DR = mybir.MatmulPerfMode.DoubleRow

B, N, D, H, DH = 2, 2048, 1024, 16, 64
NC = 8            # cores
HC = H // NC      # heads per core = 2
TOK = B * N       # 4096
CH = 8            # token chunks of 512 for projections
CW = TOK // CH    # 512
KTD = D // 128    # 8 contraction tiles for projections
NKT = N // 128    # 16 ktok tiles per batch
QC = N // 512     # 4 qchunks per batch

# int8-exp constants: u8 = trunc(s*LOG2E + MAGIC) built directly as e4m3
# bits; logits s/8 stay within +-2.2 so u8 stays in [32, 80].
EXP_MUL = float(np.log2(np.e))            # folds the 1/8 logit scale
EXP_ADD = 56.03676                        # calibrated so the int8-exp is
                                          # mean-unbiased vs the ACT path

# engine assignment patterns (A=ACT, D=DVE), tuned for makespan balance
EXPPAT = "ADADADADADADADAA"               # phase 2: A9 D7, no runs
EXPPAT3 = "ADADADADADADADAA"              # phase 3: A9 D7, no runs               # per (b,qc): 16 (ktg,h) slots
COPYPAT = "ADAADAAD"                      # A5 D3 per chunk                      # proj psum->sbuf copies per chunk


def build_nc():
    nc = bacc.Bacc(
        "TRN2",
        target_bir_lowering=False,
        debug=False,
        enable_asserts=False,
        num_devices=NC,
    )
    xT = nc.dram_tensor("xT", [D, TOK], BF16, kind="ExternalInput").ap()
    wq = nc.dram_tensor("wq", [D, 128], BF16, kind="ExternalInput").ap()
    wk = nc.dram_tensor("wk", [D, 128], BF16, kind="ExternalInput").ap()
    wv = nc.dram_tensor("wv", [D, 128], BF16, kind="ExternalInput").ap()
    wo = nc.dram_tensor("wo", [128, D], BF16, kind="ExternalInput").ap()
    bq = nc.dram_tensor("bq", [128, 1], F32, kind="ExternalInput").ap()
    out_p = nc.dram_tensor("out_p", [D, TOK], BF16, kind="ExternalOutput").ap()

    from contextlib import ExitStack

    with tile.TileContext(nc) as tc, ExitStack() as ctx:
        singles = ctx.enter_context(tc.tile_pool(name="singles", bufs=1))

        wq_sb = singles.tile([128, KTD, 128], BF16)
        nc.sync.dma_start(wq_sb, wq.rearrange("(ko p) m -> p ko m", p=128))
        wk_sb = singles.tile([128, KTD, 128], BF16)
        wv_sb = singles.tile([128, KTD, 128], BF16)
        bq_sb = singles.tile([128, 1], F32)
        wo_sb = singles.tile([128, D], BF16)

        # fp8 staging for Q/K (d-on-partitions), then DMA-restructured into
        # [32 part, dh-half, head, tok] for DoubleRow scores
        QT = [singles.tile([128, N], FP8, name=f"QT{b}") for b in range(B)]
        KT = [singles.tile([128, N], FP8, name=f"KT{b}") for b in range(B)]
        Q2 = [singles.tile([32, 2, HC, N], FP8, name=f"Q2{b}") for b in range(B)]
        K2 = [singles.tile([32, 2, HC, N], FP8, name=f"K2{b}") for b in range(B)]
        # token-major V per (b, kchunk): cols 0:64 ones (denominator rows),
        # cols 64:128 = V_h
        V1 = [[singles.tile([128, 4, HC, 2 * DH], FP8, name=f"V1{b}_{q}")
               for q in range(QC)] for b in range(B)]
        for b in range(B):
            for q in range(QC):
                nc.gpsimd.memset(V1[b][q][:, :, :, 0:DH], 1.0)
        yT = [singles.tile([128, N], BF16, name=f"yT{b}") for b in range(B)]

        # ---- Stage A: bf16 QKV projections for one 512-token chunk.
        # Emitted as a generator with 8 interleave slots so attention ktg
        # blocks can be zipped between the PE matmul groups (in-order engine
        # queues: emission order = execution order). pq occupies slots 0-3,
        # pk slots 4-7 (they share one psum bank), pv spreads k over all 8.
        def stage_a_steps(b, cc, xpool, psA, psV):
            c = b * (CH // B) + cc
            xt = xpool.tile([128, KTD, CW], BF16, tag="xt", name="xt")
            xsrc = xT[:, c * CW:(c + 1) * CW].rearrange("(ko p) n -> p ko n", p=128)
            nc.sync.dma_start(xt[:, 0:KTD // 2, :], xsrc[:, 0:KTD // 2, :])
            nc.sync.dma_start(xt[:, KTD // 2:, :], xsrc[:, KTD // 2:, :])
            if b == 0 and cc == 0:
                nc.sync.dma_start(wk_sb, wk.rearrange("(ko p) m -> p ko m", p=128))
                nc.sync.dma_start(wv_sb, wv.rearrange("(ko p) m -> p ko m", p=128))
                nc.sync.dma_start(bq_sb, bq)
                nc.sync.dma_start(wo_sb, wo)
            bs = slice(c * CW - b * N, (c + 1) * CW - b * N)
            pq = psA.tile([128, CW], F32, tag="pqk", name="pq")
            pv = psV.tile([128, 4, 128], F32, tag="pv", name="pv")
            pk = None
            for slot in range(8):
                if slot < 4:
                    for kk in (2 * slot, 2 * slot + 1):
                        nc.tensor.matmul(pq, lhsT=wq_sb[:, kk, :], rhs=xt[:, kk, :],
                                         start=(kk == 0), stop=(kk == KTD - 1))
                else:
                    if pk is None:
                        pk = psA.tile([128, CW], F32, tag="pqk", name="pk")
                    for kk in (2 * (slot - 4), 2 * (slot - 4) + 1):
                        nc.tensor.matmul(pk, lhsT=wk_sb[:, kk, :], rhs=xt[:, kk, :],
                                         start=(kk == 0), stop=(kk == KTD - 1))
                # pv s-subtiles share one psum bank; start=True flags the
                # whole 2KB region pending-zero, so each s must fully finish
                # (k 0..7) before the next s starts: s-major emission
                s = slot // 2
                for kk in range(4 * (slot % 2), 4 * (slot % 2) + 4):
                    nc.tensor.matmul(pv[:, s, :],
                                     lhsT=xt[:, kk, s * 128:(s + 1) * 128],
                                     rhs=wv_sb[:, kk, :],
                                     start=(kk == 0), stop=(kk == KTD - 1))
                if slot == 3:
                    # q bias-add + fp8 quantize on ACT (frees the pqk bank)
                    nc.scalar.activation(QT[b][:, bs], pq, AF.Identity,
                                         bias=bq_sb[:, 0:1])
                yield
            nc.vector.tensor_copy(KT[b][:, bs], pk)
            nc.scalar.copy(V1[b][cc][:, :, 0, DH:2 * DH], pv[:, :, 0:DH])
            nc.vector.tensor_copy(V1[b][cc][:, :, 1, DH:2 * DH],
                                  pv[:, :, DH:2 * DH])
            restructure_chunk(b, cc)

        def stage_a(b, cc, xpool, psA, psV):
            for _ in stage_a_steps(b, cc, xpool, psA, psV):
                pass

        # ---- per-chunk Q/K restructure to [32, half, head, tok] ----
        def restructure_chunk(b, cc):
            bs = slice(cc * CW, (cc + 1) * CW)
            for h in range(HC):
                for s in range(2):
                    ps = slice(h * 64 + 32 * s, h * 64 + 32 * (s + 1))
                    nc.sync.dma_start(Q2[b][:, s, h, bs], QT[b][ps, bs])
                    nc.sync.dma_start(K2[b][:, s, h, bs], KT[b][ps, bs])

        # ---- Output projection partial for one 512-token chunk ----
        def proj_open(b, cc, ppool):
            return ppool.tile([128, D // 128, CW], BF16, tag="ps", name="ps")

        def proj_ot(b, cc, ot, ps, psV, psA):
            cs = slice(cc * CW, (cc + 1) * CW)
            pool_, tag_ = (psV, "pv") if ot % 2 == 0 else (psA, "pqk")
            pp = pool_.tile([128, CW], F32, tag=tag_, name="pp")
            nc.tensor.matmul(pp, lhsT=wo_sb[:, ot * 128:(ot + 1) * 128],
                             rhs=yT[b][:, cs], start=True, stop=True)
            if COPYPAT[ot] == "A":
                nc.scalar.copy(ps[:, ot, :], pp)
            else:
                nc.vector.tensor_copy(ps[:, ot, :], pp)

        def proj_close(b, cc, ps):
            nc.sync.dma_start(
                out_p[:, b * N + cc * CW:b * N + (cc + 1) * CW]
                .rearrange("(o p) n -> p o n", p=128), ps)

        # ---- Attention pieces for one (batch, qchunk) ----
        def attn_open(yps):
            return [yps.tile([128, 512], F32, tag=f"y{h}", name=f"py{h}")
                    for h in range(HC)]

        def attn_se_h(b, qc, ktg, h, stp, epool, pat):
            # scores + exp for one (ktg, head); separate emission per head
            # staggers the two stt psum buffers against each other
            qs = slice(qc * 512, qc * 512 + 512)
            stt = stp.tile([128, 2, 512], F32, tag="st", name="stt")
            for j in range(2):
                kt = ktg * 2 + j
                nc.tensor.matmul(
                    stt[:, j, :],
                    lhsT=K2[b][:, :, h, kt * 128:(kt + 1) * 128],
                    rhs=Q2[b][:, :, h, qs],
                    start=True, stop=True, perf_mode=DR)
            et = epool.tile([128, 2, 512], FP8, tag="et", name="et")
            if pat[ktg * 2 + h] == "A":
                nc.scalar.activation(et, stt, AF.Exp, scale=0.125)
            else:
                nc.vector.tensor_scalar(
                    et.bitcast(U8), stt, EXP_MUL, EXP_ADD,
                    ALU.mult, ALU.add)
            return et

        def attn_av_h(b, ktg, h, py, et):
            kc, ks = divmod(ktg * 2, 4)
            nc.tensor.matmul(
                py[h],
                lhsT=V1[b][kc][:, ks:ks + 2, h, :],
                rhs=et,
                start=(ktg == 0), stop=(ktg == NKT // 2 - 1),
                perf_mode=DR)

        def attn_close(b, qc, py, epool):
            qs = slice(qc * 512, qc * 512 + 512)
            for h in range(HC):
                rsb = epool.tile([64, 512], F32, tag="den", name="rsb")
                nc.vector.reciprocal(rsb, py[h][0:DH, :])
                nc.vector.tensor_tensor(yT[b][h * DH:(h + 1) * DH, qs],
                                        py[h][DH:2 * DH, :],
                                        rsb, ALU.mult)

        with tc.tile_pool(name="xp", bufs=4) as xpool, \
             tc.tile_pool(name="psA", bufs=1, space="PSUM") as psA, \
             tc.tile_pool(name="psV", bufs=1, space="PSUM") as psV, \
             tc.tile_pool(name="stp", bufs=2, space="PSUM") as stp, \
             tc.tile_pool(name="yps", bufs=1, space="PSUM") as yps, \
             tc.tile_pool(name="ep", bufs=10) as epool, \
             tc.tile_pool(name="pp", bufs=2) as ppool:

            class AttIter:
                """One attention (batch, qchunk) iteration; emits scores/exp
                blocks on demand with the AVs auto-trailing by LAG blocks, so
                a new iteration's exp work can pre-run while the previous
                iteration's AVs and normalize drain."""

                LAG = 4   # in (ktg, h) half-blocks

                def __init__(self, b, qc, pat):
                    self.b, self.qc, self.pat = b, qc, pat
                    self.py = None
                    self.pend = []
                    self.i = 0

                def half(self):
                    if self.py is None:
                        self.py = attn_open(yps)
                    ktg, h = divmod(self.i, HC)
                    et = attn_se_h(self.b, self.qc, ktg, h, stp, epool, self.pat)
                    self.pend.append((ktg, h, et))
                    self.i += 1
                    while len(self.pend) > self.LAG:
                        k, hh, e = self.pend.pop(0)
                        attn_av_h(self.b, k, hh, self.py, e)

                def se(self, n):
                    # n counts full ktg blocks = 2 half-blocks
                    return [self.half] * (2 * n)

                blocks = se

                def close(self):
                    for k, hh, e in self.pend:
                        attn_av_h(self.b, k, hh, self.py, e)
                    self.pend = []
                    attn_close(self.b, self.qc, self.py, epool)

            class ProjIter:
                def __init__(self, b, cc):
                    self.b, self.cc = b, cc
                    self.ps = None
                    self.ot = 0

                def one(self):
                    if self.ps is None:
                        self.ps = proj_open(self.b, self.cc, ppool)
                    proj_ot(self.b, self.cc, self.ot, self.ps, psV, psA)
                    self.ot += 1

                def ots(self, n):
                    return [self.one] * n

                def close(self):
                    proj_close(self.b, self.cc, self.ps)

            def zipg(stage, work):
                """Interleave one stage chunk's 8 slots with work closures."""
                n = len(work)
                i = 0
                for s in range(8):
                    if stage is not None:
                        next(stage)
                    take = (n * (s + 1)) // 8 - (n * s) // 8
                    for _ in range(take):
                        work[i]()
                        i += 1
                if stage is not None:
                    for _ in stage:
                        pass

            att = {(b, qc): AttIter(b, qc, EXPPAT if b == 0 else EXPPAT3)
                   for b in range(B) for qc in range(QC)}
            prj = {(b, cc): ProjIter(b, cc) for b in range(B) for cc in range(QC)}
            S = lambda b, cc: stage_a_steps(b, cc, xpool, psA, psV)

            a00, a01, a02, a03 = (att[(0, q)] for q in range(QC))
            a10, a11, a12, a13 = (att[(1, q)] for q in range(QC))
            p00, p01, p02, p03 = (prj[(0, c)] for c in range(QC))
            p10, p11, p12, p13 = (prj[(1, c)] for c in range(QC))

            zipg(S(0, 0), [])
            zipg(S(0, 1), a00.se(2))
            zipg(S(0, 2), a00.se(2) + a01.se(2))
            zipg(S(0, 3), a00.se(2) + a01.se(2))
            zipg(S(1, 0), a00.se(2) + [a00.close] + a01.se(2) + a02.se(2))
            zipg(S(1, 1), a01.se(2) + [a01.close] + a02.se(2) + p00.ots(4)
                 + a03.se(1))
            zipg(S(1, 2), a02.se(4) + [a02.close] + p00.ots(4) + [p00.close]
                 + a03.se(1))
            zipg(S(1, 3), a03.se(4) + p01.ots(4) + a10.se(1))
            zipg(None, a03.se(2) + [a03.close] + p01.ots(4) + [p01.close]
                 + a10.se(5))
            zipg(None, a10.se(2) + [a10.close] + p02.ots(8) + [p02.close]
                 + a11.se(2))
            zipg(None, a11.se(6) + [a11.close] + p03.ots(8) + [p03.close]
                 + a12.se(2))
            zipg(None, a12.se(6) + [a12.close] + p10.ots(8) + [p10.close]
                 + a13.se(2))
            zipg(None, a13.se(6) + [a13.close] + p11.ots(8) + [p11.close])
            zipg(None, p12.ots(8) + [p12.close] + p13.ots(8) + [p13.close])

    nc.compile()
    return nc


_CACHE = {}


def _get_nc():
    if "nc" not in _CACHE:
        _CACHE["nc"] = build_nc()
    return _CACHE["nc"]


def _prep_inputs(x, Wqkv, bqkv, Wproj):
    bf = ml_dtypes.bfloat16
    x = np.asarray(x, np.float32)
    Wqkv = np.asarray(Wqkv, np.float32)
    bqkv = np.asarray(bqkv, np.float32)
    Wproj = np.asarray(Wproj, np.float32)
    xT = np.ascontiguousarray(x.reshape(TOK, D).T).astype(bf)
    in_maps = []
    for c in range(NC):
        m = {
            "xT": xT,
            "wq": np.ascontiguousarray(
                Wqkv[:, 0 * D + c * 128:0 * D + (c + 1) * 128]).astype(bf),
            "wk": np.ascontiguousarray(
                Wqkv[:, 1 * D + c * 128:1 * D + (c + 1) * 128]).astype(bf),
            "wv": np.ascontiguousarray(
                Wqkv[:, 2 * D + c * 128:2 * D + (c + 1) * 128]).astype(bf),
            "bq": np.ascontiguousarray(
                bqkv[c * 128:(c + 1) * 128]).reshape(128, 1).astype(np.float32),
            "wo": np.ascontiguousarray(Wproj[c * 128:(c + 1) * 128, :]).astype(bf),
        }
        in_maps.append(m)
    return in_maps


def _run(x, Wqkv, bqkv, Wproj, bproj, trace=False):
    Wproj = np.asarray(Wproj, np.float32)
    bproj = np.asarray(bproj, np.float32)
    bqkv_np = np.asarray(bqkv, np.float32)
    in_maps = _prep_inputs(x, Wqkv, bqkv_np, Wproj)
    nc = _get_nc()
    res = run_bass_kernel_spmd(nc, in_maps, core_ids=list(range(NC)), trace=trace)
    acc = res.results[0]["out_p"].astype(np.float32).copy()
    for c in range(1, NC):
        acc += res.results[c]["out_p"].astype(np.float32)
    bv = bqkv_np[2 * D:]
    bias_eff = (bv @ Wproj + bproj).astype(np.float32)
    out = np.ascontiguousarray(acc.T).reshape(B, N, D) + bias_eff
    return out.astype(np.float32), res


def kernel(x, Wqkv, bqkv, Wproj, bproj):
    out, _ = _run(x, Wqkv, bqkv, Wproj, bproj, trace=False)
    return out
